# revision 1
# baseline (speedup 1.0000x reference)
"""Trainium2 Bass kernel for pairwise Tang-Toennies dispersion energy.

Problem: for P=3.2M random atom pairs over N=100k atoms in a periodic box,
    ene[p] = -(c6[ti,tj] * f6(b[ti,tj]*r)) / r^6   if r <= cutoff else 0
with r the minimum-image distance and f6 the 6th-order Tang-Toennies damper.

The TRN2 indirect-DMA path only honours one dynamic offset per SBUF
partition per instruction (128 gathered rows/instruction), which is far too
slow for 6.4M row fetches, so this kernel avoids per-pair gathers entirely
with a degree-grouped layout:

  host:   group pairs by each endpoint into fixed-size R-slot rows of a
          "virtual atom" list (heavy atoms get several rows, every row has
          exactly R pair slots, pads unused).  All index math (sorting,
          slot maps, the A->B slot permutation) is host-side numpy.
  pass 1: (j-grouped) device expands the replicated atom table into one
          16B record (x,y,z,type) per j-slot via 0-stride broadcast DMA —
          pure streaming, no indirection.
  host:   permutes pass-1 records from j-slot order to i-slot order
          (pure routing of device-produced bytes).
  pass 2: (i-grouped) device streams the aligned j-records plus a
          sequential per-row window of the i-side table (coords + the
          c6/b table rows for the atom's type), selects the tj column of
          c6/b with a 3-level mask tree, and evaluates the energy.
          Minimum image uses min(|d|, L-|d|) (exactly equal to the
          reference's round() formulation).  r and 1/r^6 come from one
          Ln + two Exp so a single activation-table set suffices.
  host:   scatters per-slot energies back to the original pair order.

Work is sharded by virtual-atom rows: every core gets the same number of
rows (and therefore slots), so the 8 cores are perfectly balanced.
"""

import numpy as np

import concourse.bacc as bacc
import concourse.bass as bass
import concourse.mybir as mybir
from concourse.tile import TileContext
from concourse.bass_utils import run_bass_kernel_spmd

F32 = mybir.dt.float32
I32 = mybir.dt.int32
U8 = mybir.dt.uint8
AF = mybir.ActivationFunctionType
OP = mybir.AluOpType

N_CORES = 8
R = 40            # pair slots per virtual atom row
WA = 16           # virtual rows per partition per tile
TILE_VROWS = 128 * WA
K = WA * R        # pair slots per partition per tile


PLANES = ("jx", "jy", "jz", "jt")


def build_pass1(vrows_core, reps=1, t_limit=None):
    """Expand slim rows [v,4] into 4 SoA planes [v*R] (row repeated R times).

    The expansion itself runs on DVE via 0-stride reads (free); the DMAs are
    all dense/contiguous.
    """
    import contextlib
    nc = bacc.Bacc(trn_type="TRN2", target_bir_lowering=False)
    WA1 = 8
    TV1 = 128 * WA1
    K1 = WA1 * R
    slimv = nc.dram_tensor("slimv", [vrows_core, 4], F32, kind="ExternalInput")
    planes = {n: nc.dram_tensor(n, [vrows_core * R], F32, kind="ExternalOutput")
              for n in PLANES}
    T = vrows_core // TV1
    if t_limit is not None:
        T = min(T, t_limit)
    with TileContext(nc) as tc:
        rep_ctx = tc.For_i(0, reps, 1) if reps > 1 else contextlib.nullcontext()
        with tc.tile_pool(name="w", bufs=3) as pool, rep_ctx:
            for t in range(T):
                w = pool.tile([128, WA1 * 4], F32, tag="w")
                nc.sync.dma_start(
                    out=w[:],
                    in_=slimv[bass.ts(t, TV1)].rearrange(
                        "(p a) d -> p (a d)", a=WA1),
                )
                for ci, n in enumerate(PLANES):
                    e = pool.tile([128, K1], F32, tag=f"e{n}")
                    nc.vector.tensor_copy(
                        e[:].rearrange("p (a r) -> p a r", a=WA1),
                        bass.AP(w.tensor, ci, [w[:].ap[0], [4, WA1], [0, R]]),
                    )
                    nc.sync.dma_start(
                        out=planes[n][bass.ts(t, TV1 * R)].rearrange(
                            "(p x) -> p x", x=K1),
                        in_=e[:],
                    )
    nc.compile()
    return nc


def build_pass2(vrows_core, Ls, cutoff, reps=1, t_limit=None):
    """i-grouped energy kernel. ilite rows: x,y,z,pad,c6row[8],b_row[8],pad.

    reps>1 re-runs the whole tile loop inside a hardware loop (same data,
    same output) — used only for slope-based wall-clock timing.
    """
    nc = bacc.Bacc(trn_type="TRN2", target_bir_lowering=False)
    ilite = nc.dram_tensor("ilite", [vrows_core, 20], F32, kind="ExternalInput")
    planes = {n: nc.dram_tensor(n, [vrows_core * R], F32, kind="ExternalInput")
              for n in PLANES}
    ene_d = nc.dram_tensor("ene", [vrows_core * R], F32, kind="ExternalOutput")
    T = vrows_core // TILE_VROWS
    if t_limit is not None:
        T = min(T, t_limit)
    c2 = float(np.float32(cutoff) ** 2)

    import contextlib

    with TileContext(nc) as tc:
        rep_ctx = tc.For_i(0, reps, 1) if reps > 1 else contextlib.nullcontext()
        with tc.tile_pool(name="io", bufs=2) as pio, \
             tc.tile_pool(name="hot", bufs=2) as phot, \
             tc.tile_pool(name="work", bufs=1) as pool, rep_ctx:
            for t in range(T):
                w = pio.tile([128, WA * 20], F32, tag="w")
                nc.sync.dma_start(
                    out=w[:],
                    in_=ilite[bass.ts(t, TILE_VROWS)].rearrange(
                        "(p a) d -> p (a d)", a=WA),
                )
                jp = {}
                for n in PLANES:
                    jt_ = pio.tile([128, K], F32, tag=f"j{n}")
                    nc.sync.dma_start(
                        out=jt_[:],
                        in_=planes[n][bass.ts(t, TILE_VROWS * R)].rearrange(
                            "(p x) -> p x", x=K),
                    )
                    jp[n] = jt_
                wp = w[:].ap[0]

                def wbc(off):
                    # window scalar broadcast to [128, WA, R] (flat K)
                    return bass.AP(w.tensor, off, [wp, [20, WA], [0, R]])

                def wbc4(off, step, cnt):
                    # window vector broadcast to [128, WA, R, cnt]
                    return bass.AP(w.tensor, off, [wp, [20, WA], [0, R], [step, cnt]])

                # ---- type bits of tj -> masks (gpsimd) ----
                tji = pool.tile([128, K], I32, tag="tji")
                nc.gpsimd.tensor_copy(tji[:], jp["jt"][:])
                masks = []
                for bit in range(3):
                    mi = pool.tile([128, K], I32, tag=f"mi{bit}")
                    nc.vector.tensor_scalar(
                        out=mi[:], in0=tji[:], scalar1=bit, scalar2=1,
                        op0=OP.logical_shift_right, op1=OP.bitwise_and)
                    mb = pool.tile([128, K], U8, tag=f"mb{bit}")
                    nc.gpsimd.tensor_copy(mb[:], mi[:])
                    masks.append(mb)

                def mask4(mb, cnt):
                    return bass.AP(mb.tensor, 0, [mb[:].ap[0], [R, WA], [1, R], [0, cnt]])

                # ---- fused select tree: c6 and b columns together (DVE)
                # ilite cols 4..19 = c6row[8] ++ brow[8]; odd/even pairs of
                # both tables are selected in one instruction per level.
                s1 = pool.tile([128, K * 8], F32, tag="s1")
                nc.vector.select(
                    out=bass.AP(s1.tensor, 0, [s1[:].ap[0], [R * 8, WA], [8, R], [1, 8]]),
                    mask=mask4(masks[0], 8),
                    on_true=wbc4(5, 2, 8),
                    on_false=wbc4(4, 2, 8),
                )
                s2 = pool.tile([128, K * 4], F32, tag="s2")
                nc.vector.select(
                    out=bass.AP(s2.tensor, 0, [s2[:].ap[0], [R * 4, WA], [4, R], [1, 4]]),
                    mask=mask4(masks[1], 4),
                    on_true=bass.AP(s1.tensor, 1, [s1[:].ap[0], [R * 8, WA], [8, R], [2, 4]]),
                    on_false=bass.AP(s1.tensor, 0, [s1[:].ap[0], [R * 8, WA], [8, R], [2, 4]]),
                )
                s3 = pool.tile([128, K * 2], F32, tag="s3")
                nc.vector.select(
                    out=bass.AP(s3.tensor, 0, [s3[:].ap[0], [R * 2, WA], [2, R], [1, 2]]),
                    mask=mask4(masks[2], 2),
                    on_true=bass.AP(s2.tensor, 1, [s2[:].ap[0], [R * 4, WA], [4, R], [2, 2]]),
                    on_false=bass.AP(s2.tensor, 0, [s2[:].ap[0], [R * 4, WA], [4, R], [2, 2]]),
                )
                s3v = s3[:].rearrange("p (k two) -> p k two", two=2)
                sel = {"c6": None, "b": None}

                # ---- minimum image + r2 ----
                sq = []
                for ax, L in enumerate(Ls):
                    d_ax = pool.tile([128, K], F32, tag=f"d{ax}")
                    nc.vector.tensor_tensor(
                        out=d_ax[:].rearrange("p (a r) -> p a r", a=WA),
                        in0=jp[PLANES[ax]][:].rearrange("p (a r) -> p a r", a=WA),
                        in1=wbc(ax), op=OP.subtract)
                    ax_t = pool.tile([128, K], F32, tag=f"a{ax}")
                    nc.scalar.activation(ax_t[:], d_ax[:], AF.Abs)
                    wd = pool.tile([128, K], F32, tag=f"wd{ax}")
                    nc.gpsimd.tensor_scalar(
                        out=wd[:], in0=ax_t[:], scalar1=-1.0, scalar2=float(L),
                        op0=OP.mult, op1=OP.add)
                    mm = pool.tile([128, K], F32, tag=f"mm{ax}")
                    nc.vector.tensor_tensor(out=mm[:], in0=ax_t[:], in1=wd[:], op=OP.min)
                    sq_t = phot.tile([128, K], F32, tag=f"sq{ax}")
                    nc.scalar.activation(sq_t[:], mm[:], AF.Square)
                    sq.append(sq_t)

                r2 = phot.tile([128, K], F32, tag="r2")
                nc.gpsimd.tensor_tensor(out=r2[:], in0=sq[0][:], in1=sq[1][:], op=OP.add)
                nc.gpsimd.tensor_tensor(out=r2[:], in0=r2[:], in1=sq[2][:], op=OP.add)

                lr2 = phot.tile([128, K], F32, tag="lr2")
                nc.scalar.activation(lr2[:], r2[:], AF.Ln)
                rr = phot.tile([128, K], F32, tag="rr")
                nc.scalar.activation(rr[:], lr2[:], AF.Exp, scale=0.5)
                ir6 = phot.tile([128, K], F32, tag="ir6")
                nc.scalar.activation(ir6[:], lr2[:], AF.Exp, scale=-3.0)

                u = phot.tile([128, K], F32, tag="u")
                nc.vector.tensor_tensor(out=u[:], in0=s3v[:, :, 1], in1=rr[:], op=OP.mult)
                em = phot.tile([128, K], F32, tag="em")
                nc.scalar.activation(em[:], u[:], AF.Exp, scale=-1.0)

                # poly = 1 + u*(1 + u/2*(1 + u/3*(1 + u/4*(1 + u/5*(1 + u/6)))))
                p = pool.tile([128, K], F32, tag="p")
                q = pool.tile([128, K], F32, tag="q")
                nc.vector.tensor_scalar(
                    out=p[:], in0=u[:], scalar1=1.0 / 6.0, scalar2=1.0,
                    op0=OP.mult, op1=OP.add)
                for k in (5, 4, 3, 2, 1):
                    nc.vector.scalar_tensor_tensor(
                        out=q[:], in0=p[:], scalar=1.0 / k, in1=u[:],
                        op0=OP.mult, op1=OP.mult)
                    nc.vector.tensor_scalar(
                        out=p[:], in0=q[:], scalar1=1.0, scalar2=None, op0=OP.add)

                # small-u branch: f6 = em*u^7/5040 * S(u),
                # S = 1 + u/8 + u^2/72 + u^3/720 + u^4/7920  (exact, no 1-x
                # cancellation; the direct form is pure rounding noise there)
                u2 = pool.tile([128, K], F32, tag="u2")
                nc.gpsimd.tensor_tensor(out=u2[:], in0=u[:], in1=u[:], op=OP.mult)
                u3 = pool.tile([128, K], F32, tag="u3")
                nc.gpsimd.tensor_tensor(out=u3[:], in0=u2[:], in1=u[:], op=OP.mult)
                u6 = pool.tile([128, K], F32, tag="u6")
                nc.gpsimd.tensor_tensor(out=u6[:], in0=u3[:], in1=u3[:], op=OP.mult)
                nc.gpsimd.tensor_tensor(out=u6[:], in0=u6[:], in1=u[:], op=OP.mult)
                S = pool.tile([128, K], F32, tag="S")
                sq_ = pool.tile([128, K], F32, tag="Sq")
                nc.gpsimd.tensor_scalar(
                    out=S[:], in0=u[:], scalar1=1.0 / 7920.0, scalar2=1.0 / 720.0,
                    op0=OP.mult, op1=OP.add)
                for ck in (1.0 / 72.0, 1.0 / 8.0, 1.0):
                    nc.gpsimd.tensor_tensor(out=sq_[:], in0=S[:], in1=u[:], op=OP.mult)
                    nc.gpsimd.tensor_scalar(
                        out=S[:], in0=sq_[:], scalar1=ck, scalar2=None, op0=OP.add)
                nc.gpsimd.tensor_scalar(
                    out=u6[:], in0=u6[:], scalar1=1.0 / 5040.0, scalar2=None,
                    op0=OP.mult)
                nc.gpsimd.tensor_tensor(out=u6[:], in0=u6[:], in1=S[:], op=OP.mult)

                # ene = A*em*poly - A with A = c6/r^6; then cutoff on r2
                A = pool.tile([128, K], F32, tag="A")
                nc.vector.tensor_tensor(out=A[:], in0=s3v[:, :, 0], in1=ir6[:], op=OP.mult)
                B = pool.tile([128, K], F32, tag="B")
                nc.vector.tensor_tensor(out=B[:], in0=A[:], in1=em[:], op=OP.mult)
                nc.vector.tensor_tensor(out=em[:], in0=B[:], in1=p[:], op=OP.mult)
                nc.vector.tensor_tensor(out=em[:], in0=em[:], in1=A[:], op=OP.subtract)
                es = pool.tile([128, K], F32, tag="es")
                nc.vector.scalar_tensor_tensor(
                    out=es[:], in0=B[:], scalar=-1.0, in1=u6[:],
                    op0=OP.mult, op1=OP.mult)
                mu = pool.tile([128, K], U8, tag="mu")
                nc.vector.tensor_scalar(
                    out=mu[:], in0=u[:], scalar1=1.2, scalar2=None, op0=OP.is_le)
                nc.vector.select(out=em[:], mask=mu[:], on_true=es[:], on_false=em[:])
                eout = pio.tile([128, K], F32, tag="eout")
                nc.vector.scalar_tensor_tensor(
                    out=eout[:], in0=r2[:], scalar=c2, in1=em[:],
                    op0=OP.is_le, op1=OP.mult)
                nc.sync.dma_start(
                    out=ene_d[bass.ts(t, TILE_VROWS * R)].rearrange("(p x) -> p x", x=K),
                    in_=eout[:])
    nc.compile()
    return nc


def _slot_map(atom_of_pair, n_atoms):
    """Return (slot per pair, virtual-row atom ids, V) for one endpoint role."""
    P = atom_of_pair.shape[0]
    d = np.bincount(atom_of_pair, minlength=n_atoms)
    rows = -(-d // R)  # ceil; 0 for degree-0 atoms
    vbase = np.zeros(n_atoms + 1, np.int64)
    np.cumsum(rows, out=vbase[1:])
    V = int(vbase[-1])
    v_atom = np.repeat(np.arange(n_atoms, dtype=np.int64), rows)
    order = np.argsort(atom_of_pair, kind="stable")
    pb = np.zeros(n_atoms + 1, np.int64)
    np.cumsum(d, out=pb[1:])
    a_sorted = atom_of_pair[order]
    rank = np.arange(P, dtype=np.int64) - pb[a_sorted]
    slot_sorted = (vbase[a_sorted] + rank // R) * R + rank % R
    slot = np.empty(P, np.int64)
    slot[order] = slot_sorted
    return slot, v_atom, V


def _pad_vrows(V):
    per_core = -(-(V + 1) // (TILE_VROWS * N_CORES)) * TILE_VROWS
    return per_core * N_CORES, per_core


_NC_CACHE = {}


def _get_nc(builder, key, *args):
    if key not in _NC_CACHE:
        _NC_CACHE[key] = builder(*args)
    return _NC_CACHE[key]


def _host_reference(coords, pairs, box, c6, b, cutoff, atom_types):
    # numpy fallback for non-orthorhombic boxes (not hit by the real inputs)
    dr = coords[pairs[:, 1]] - coords[pairs[:, 0]]
    inv_box = np.linalg.inv(box)
    dr = dr - np.round(dr @ inv_box) @ box
    r = np.sqrt((dr * dr).sum(1))
    ti = atom_types[pairs[:, 0]]
    tj = atom_types[pairs[:, 1]]
    u = b[ti, tj] * r
    poly = 1.0 + u * (1.0 + u / 2.0 * (1.0 + u / 3.0 * (1.0 + u / 4.0 *
                     (1.0 + u / 5.0 * (1.0 + u / 6.0)))))
    f6 = 1.0 - np.exp(-u) * poly
    ene = -(c6[ti, tj] * f6) / r ** 6
    return np.where(r <= cutoff, ene, 0.0).astype(np.float32)


def kernel(coords, pairs, box, c6, b, cutoff, atom_types):
    coords = np.asarray(coords, np.float32)
    pairs = np.asarray(pairs)
    box = np.asarray(box, np.float32)
    c6 = np.asarray(c6, np.float32)
    b = np.asarray(b, np.float32)
    atom_types = np.asarray(atom_types).astype(np.int64)
    cutoff = float(np.asarray(cutoff))

    offdiag = box - np.diag(np.diag(box))
    if np.any(offdiag != 0.0):
        return _host_reference(coords, pairs, box, c6, b, cutoff, atom_types)
    Ls = tuple(float(box[i, i]) for i in range(3))

    n_atoms = coords.shape[0]
    pi = np.ascontiguousarray(pairs[:, 0]).astype(np.int64)
    pj = np.ascontiguousarray(pairs[:, 1]).astype(np.int64)

    # slot maps for both roles
    sB, vi_atom, Vi = _slot_map(pi, n_atoms)
    sA, vj_atom, Vj = _slot_map(pj, n_atoms)
    Vi_pad, vi_core = _pad_vrows(Vi)
    Vj_pad, vj_core = _pad_vrows(Vj)

    # pass-1 input table: slim rows per virtual j row (+ far-away dummies)
    slimv = np.empty((Vj_pad, 4), np.float32)
    slimv[:Vj, :3] = coords[vj_atom]
    slimv[:Vj, 3] = atom_types[vj_atom]
    slimv[Vj:] = (1e4, 1e4, 1e4, 0.0)

    # pass-2 window table: x,y,z,pad,c6row[8],brow[8],pad per virtual i row
    ilite = np.empty((Vi_pad, 20), np.float32)
    ilite[:Vi, 0:3] = coords[vi_atom]
    ilite[:Vi, 3] = 0.0
    ti_v = atom_types[vi_atom]
    ilite[:Vi, 4:12] = c6[ti_v]
    ilite[:Vi, 12:20] = b[ti_v]
    ilite[Vi:, 0:3] = 2e4
    ilite[Vi:, 3:] = 1.0

    # ---- pass 1: expand j-records on device ----
    nc1 = _get_nc(build_pass1, ("p1", vj_core), vj_core)
    in1 = [{"slimv": slimv[c * vj_core:(c + 1) * vj_core]} for c in range(N_CORES)]
    res1 = run_bass_kernel_spmd(nc1, in1, core_ids=list(range(N_CORES)))
    plane_A = {n: np.concatenate([res1.results[c][n] for c in range(N_CORES)])
               for n in PLANES}

    # ---- host routing: A-slot order -> B-slot order ----
    a_of_b = np.full(Vi_pad * R, Vj_pad * R - 1, np.int64)  # default: dummy j row
    a_of_b[sB] = sA
    plane_B = {n: plane_A[n][a_of_b] for n in PLANES}

    # ---- pass 2: energies per B slot ----
    nc2 = _get_nc(build_pass2, ("p2", vi_core, Ls, round(cutoff, 6)),
                  vi_core, Ls, cutoff)
    sc = vi_core * R
    in2 = [dict(ilite=ilite[c * vi_core:(c + 1) * vi_core],
                **{n: plane_B[n][c * sc:(c + 1) * sc] for n in PLANES})
           for c in range(N_CORES)]
    res2 = run_bass_kernel_spmd(nc2, in2, core_ids=list(range(N_CORES)))
    ene_B = np.concatenate([res2.results[c]["ene"] for c in range(N_CORES)])

    return ene_B[sB].astype(np.float32)



# revision 5
# speedup vs baseline: 29.0065x; 29.0065x over previous
"""Trainium2 Bass kernel for pairwise Tang-Toennies dispersion energy.

Problem: for P=3.2M random atom pairs over N=100k atoms in a periodic box,
    ene[p] = -(c6[ti,tj] * f6(b[ti,tj]*r)) / r^6   if r <= cutoff else 0
with r the minimum-image distance and f6 the 6th-order Tang-Toennies damper.

Only ~2% of random pairs fall inside the 10A cutoff, so the kernel is split
into a cheap dense screen and an exact sparse evaluation:

  host:     normalize coords to box units (x/L, an O(N) table prep; the
            i-side planes are negated so the device subtract is an add) and
            gather per-pair SoA planes jx..jz / ix..iz (pure index routing).
  kernel A: (dense, 3.2M slots) minimum-image r^2 per axis via
            m = min(|d|, 1-|d|) (exactly equal to the reference's round()
            form for |d| < 1), then a u8 in-cutoff mask.  Abs/Square run on
            the Act engine from a single activation-table set, so the hot
            loop has no table switches.
  host:     compacts the device-produced mask (np.flatnonzero + index
            gathers -- no host float math decides anything).
  kernel B: (sparse, ~62k slots) full energy: r^2 recomputed identically,
            r and r^-6 via Ln/Exp, Tang-Toennies f6 via an Estrin-form
            polynomial, exact series fallback for r^2<=0.4 where the
            1 - e^-u*poly form is pure f32 cancellation noise.
  host:     scatters the survivor energies into the zero-filled output.

Work is sharded by slots: every core gets the same slot count for both
kernels, so the 8 cores are perfectly balanced.
"""

import contextlib

import numpy as np

import concourse.bacc as bacc
import concourse.bass as bass
import concourse.mybir as mybir
from concourse.tile import TileContext
from concourse.bass_utils import run_bass_kernel_spmd

F32 = mybir.dt.float32
U8 = mybir.dt.uint8
AF = mybir.ActivationFunctionType
OP = mybir.AluOpType

N_CORES = 8

# kernel A (dense screen) tiling: slots/core = 128 * KA * TA
KA = 1042
TA = 3
SLOTS_A = 128 * KA * TA          # 400,128 per core

# kernel B (sparse energy) tiling
KB = 80
TB = 1
SLOTS_B = 128 * KB * TB          # 10,240 per core (81,920 total capacity)

AXES = ("x", "y", "z")


def _geometry(nc, pw, jt, it, Ls, K):
    """Per-axis minimum-image L^2 m^2 then r^2.  `it` holds NEGATED coords.

    m = min(|d|, 1-|d|) with d = j + (-i); Square applies the box scale.
    """
    sq = []
    for ci, ax in enumerate(AXES):
        d = pw.tile([128, K], F32, tag=f"d{ax}", name=f"d{ax}")
        nc.gpsimd.tensor_tensor(out=d[:], in0=jt[ax][:], in1=it[ax][:],
                                op=OP.add)
        a1 = pw.tile([128, K], F32, tag=f"a{ax}", name=f"a1{ax}")
        nc.scalar.activation(a1[:], d[:], AF.Abs)
        w1 = pw.tile([128, K], F32, tag=f"w{ax}", name=f"w1{ax}")
        nc.vector.tensor_scalar(out=w1[:], in0=a1[:], scalar1=-1.0,
                                scalar2=1.0, op0=OP.mult, op1=OP.add)
        m = pw.tile([128, K], F32, tag=f"m{ax}", name=f"m{ax}")
        nc.vector.tensor_tensor(out=m[:], in0=a1[:], in1=w1[:], op=OP.min)
        s = pw.tile([128, K], F32, tag=f"s{ax}", name=f"s{ax}")
        nc.scalar.activation(s[:], m[:], AF.Square, scale=float(Ls[ci]))
        sq.append(s)
    r2 = pw.tile([128, K], F32, tag="r2", name="r2")
    nc.vector.tensor_tensor(out=r2[:], in0=sq[0][:], in1=sq[1][:], op=OP.add)
    nc.gpsimd.tensor_tensor(out=r2[:], in0=r2[:], in1=sq[2][:], op=OP.add)
    return r2


def build_a(Ls, cutoff, reps=1, t_limit=None):
    """Dense screen: per-slot minimum-image r^2 -> u8 (r2 <= cutoff^2)."""
    nc = bacc.Bacc(trn_type="TRN2", target_bir_lowering=False)
    jp = {ax: nc.dram_tensor(f"j{ax}", [SLOTS_A], F32, kind="ExternalInput")
          for ax in AXES}
    ip = {ax: nc.dram_tensor(f"i{ax}", [SLOTS_A], F32, kind="ExternalInput")
          for ax in AXES}
    mask_d = nc.dram_tensor("mask", [SLOTS_A], U8, kind="ExternalOutput")
    c2 = float(np.float32(cutoff) ** 2)
    T = TA if t_limit is None else min(TA, t_limit)

    with TileContext(nc) as tc:
        rep_ctx = tc.For_i(0, reps, 1) if reps > 1 else contextlib.nullcontext()
        with tc.tile_pool(name="io", bufs=2) as pio, \
             tc.tile_pool(name="work", bufs=2) as pw, rep_ctx:
            for t in range(T):
                jt, it = {}, {}
                for ax in AXES:
                    jt[ax] = pio.tile([128, KA], F32, tag=f"j{ax}",
                                      name=f"jt{ax}")
                    nc.sync.dma_start(
                        out=jt[ax][:],
                        in_=jp[ax][bass.ts(t, 128 * KA)].rearrange(
                            "(p x) -> p x", x=KA))
                    it[ax] = pio.tile([128, KA], F32, tag=f"i{ax}",
                                      name=f"it{ax}")
                    nc.sync.dma_start(
                        out=it[ax][:],
                        in_=ip[ax][bass.ts(t, 128 * KA)].rearrange(
                            "(p x) -> p x", x=KA))
                r2 = _geometry(nc, pw, jt, it, Ls, KA)
                mk = pio.tile([128, KA], U8, tag="mk")
                nc.vector.tensor_scalar(out=mk[:], in0=r2[:], scalar1=c2,
                                        scalar2=None, op0=OP.is_le)
                nc.sync.dma_start(
                    out=mask_d[bass.ts(t, 128 * KA)].rearrange(
                        "(p x) -> p x", x=KA),
                    in_=mk[:])
    nc.compile()
    return nc


def build_b(Ls, cutoff, reps=1):
    """Sparse exact energy on compacted in-cutoff slots.

    Inputs: negated-i coord planes, cp = -c6[ti,tj], bp = b[ti,tj].
    """
    nc = bacc.Bacc(trn_type="TRN2", target_bir_lowering=False)
    jp = {ax: nc.dram_tensor(f"j{ax}", [SLOTS_B], F32, kind="ExternalInput")
          for ax in AXES}
    ip = {ax: nc.dram_tensor(f"i{ax}", [SLOTS_B], F32, kind="ExternalInput")
          for ax in AXES}
    cp_d = nc.dram_tensor("cp", [SLOTS_B], F32, kind="ExternalInput")
    bp_d = nc.dram_tensor("bp", [SLOTS_B], F32, kind="ExternalInput")
    ene_d = nc.dram_tensor("ene", [SLOTS_B], F32, kind="ExternalOutput")
    c2 = float(np.float32(cutoff) ** 2)
    K = KB

    with TileContext(nc) as tc:
        rep_ctx = tc.For_i(0, reps, 1) if reps > 1 else contextlib.nullcontext()
        with tc.tile_pool(name="io", bufs=2) as pio, \
             tc.tile_pool(name="work", bufs=2) as pw, rep_ctx:
            for t in range(TB):
                jt, it = {}, {}
                for ax in AXES:
                    jt[ax] = pio.tile([128, K], F32, tag=f"j{ax}",
                                      name=f"jt{ax}")
                    nc.sync.dma_start(
                        out=jt[ax][:],
                        in_=jp[ax][bass.ts(t, 128 * K)].rearrange(
                            "(p x) -> p x", x=K))
                    it[ax] = pio.tile([128, K], F32, tag=f"i{ax}",
                                      name=f"it{ax}")
                    nc.sync.dma_start(
                        out=it[ax][:],
                        in_=ip[ax][bass.ts(t, 128 * K)].rearrange(
                            "(p x) -> p x", x=K))
                cp = pio.tile([128, K], F32, tag="cp")
                nc.sync.dma_start(
                    out=cp[:],
                    in_=cp_d[bass.ts(t, 128 * K)].rearrange("(p x) -> p x", x=K))
                bp = pio.tile([128, K], F32, tag="bp")
                nc.sync.dma_start(
                    out=bp[:],
                    in_=bp_d[bass.ts(t, 128 * K)].rearrange("(p x) -> p x", x=K))

                # ---- geometry: identical arithmetic to kernel A ----
                r2 = _geometry(nc, pw, jt, it, Ls, K)

                # ---- r, r^-6 via one Ln + two Exp ----
                lr2 = pw.tile([128, K], F32, tag="lr2")
                nc.scalar.activation(lr2[:], r2[:], AF.Ln)
                rr = pw.tile([128, K], F32, tag="rr")
                nc.scalar.activation(rr[:], lr2[:], AF.Exp, scale=0.5)
                ir6 = pw.tile([128, K], F32, tag="ir6")
                nc.scalar.activation(ir6[:], lr2[:], AF.Exp, scale=-3.0)

                u = pw.tile([128, K], F32, tag="u")
                nc.vector.tensor_tensor(out=u[:], in0=bp[:], in1=rr[:],
                                        op=OP.mult)
                em = pw.tile([128, K], F32, tag="em")
                nc.scalar.activation(em[:], u[:], AF.Exp, scale=-1.0)
                w = pw.tile([128, K], F32, tag="w")
                nc.scalar.activation(w[:], u[:], AF.Square)

                # ---- poly(u) = sum_0^6 u^k/k!, Estrin form:
                # (1+u) + w*(1/2 + u/6) + w^2*((1/24 + u/120) + w/720)
                a_ = pw.tile([128, K], F32, tag="a_")
                nc.vector.tensor_scalar(out=a_[:], in0=u[:], scalar1=1.0 / 6.0,
                                        scalar2=0.5, op0=OP.mult, op1=OP.add)
                bq = pw.tile([128, K], F32, tag="bq")
                nc.gpsimd.tensor_scalar(out=bq[:], in0=u[:], scalar1=1.0 / 120.0,
                                        scalar2=1.0 / 24.0, op0=OP.mult,
                                        op1=OP.add)
                d2 = pw.tile([128, K], F32, tag="d2")
                nc.vector.scalar_tensor_tensor(out=d2[:], in0=w[:],
                                               scalar=1.0 / 720.0, in1=bq[:],
                                               op0=OP.mult, op1=OP.add)
                t1 = pw.tile([128, K], F32, tag="t1")
                nc.gpsimd.tensor_scalar(out=t1[:], in0=u[:], scalar1=1.0,
                                        scalar2=None, op0=OP.add)
                g = pw.tile([128, K], F32, tag="g")
                nc.vector.tensor_tensor(out=g[:], in0=w[:], in1=d2[:],
                                        op=OP.mult)
                nc.gpsimd.tensor_tensor(out=g[:], in0=g[:], in1=a_[:],
                                        op=OP.add)
                nc.vector.tensor_tensor(out=g[:], in0=w[:], in1=g[:],
                                        op=OP.mult)
                p = pw.tile([128, K], F32, tag="p")
                nc.gpsimd.tensor_tensor(out=p[:], in0=g[:], in1=t1[:],
                                        op=OP.add)

                # ---- ene = B'*poly - A' with A' = c6/r^6.  cp = -c6, so
                # An = -A', Bn = An*em = -B', ene = (-1*Bn)*poly + An. ----
                An = pw.tile([128, K], F32, tag="An")
                nc.vector.tensor_tensor(out=An[:], in0=cp[:], in1=ir6[:],
                                        op=OP.mult)
                Bn = pw.tile([128, K], F32, tag="Bn")
                nc.gpsimd.tensor_tensor(out=Bn[:], in0=An[:], in1=em[:],
                                        op=OP.mult)
                ene = pw.tile([128, K], F32, tag="ene")
                nc.vector.scalar_tensor_tensor(out=ene[:], in0=Bn[:],
                                               scalar=-1.0, in1=p[:],
                                               op0=OP.mult, op1=OP.mult)
                nc.gpsimd.tensor_tensor(out=ene[:], in0=ene[:], in1=An[:],
                                        op=OP.add)

                # ---- small-u exact series: f6 = em*u^7/5040*(1+u/8+u^2/72)
                # (the direct 1-em*poly form is f32 cancellation noise there;
                # es = (Bn/5040)*u7*S = -B'/5040*u7*S, correctly negative)
                u3 = pw.tile([128, K], F32, tag="u3")
                nc.vector.tensor_tensor(out=u3[:], in0=u[:], in1=w[:],
                                        op=OP.mult)
                u6 = pw.tile([128, K], F32, tag="u6")
                nc.scalar.activation(u6[:], u3[:], AF.Square)
                u7 = pw.tile([128, K], F32, tag="u7")
                nc.gpsimd.tensor_tensor(out=u7[:], in0=u6[:], in1=u[:],
                                        op=OP.mult)
                s1u = pw.tile([128, K], F32, tag="s1u")
                nc.vector.tensor_scalar(out=s1u[:], in0=u[:], scalar1=1.0 / 8.0,
                                        scalar2=1.0, op0=OP.mult, op1=OP.add)
                S = pw.tile([128, K], F32, tag="S")
                nc.vector.scalar_tensor_tensor(out=S[:], in0=w[:],
                                               scalar=1.0 / 72.0, in1=s1u[:],
                                               op0=OP.mult, op1=OP.add)
                es = pw.tile([128, K], F32, tag="es")
                nc.gpsimd.tensor_tensor(out=es[:], in0=u7[:], in1=S[:],
                                        op=OP.mult)
                nc.vector.scalar_tensor_tensor(out=es[:], in0=Bn[:],
                                               scalar=1.0 / 5040.0, in1=es[:],
                                               op0=OP.mult, op1=OP.mult)
                mu = pw.tile([128, K], U8, tag="mu")
                nc.vector.tensor_scalar(out=mu[:], in0=r2[:], scalar1=0.4,
                                        scalar2=None, op0=OP.is_le)
                nc.vector.select(out=ene[:], mask=mu[:], on_true=es[:],
                                 on_false=ene[:])

                # ---- cutoff (identical compare to kernel A's mask) ----
                eout = pio.tile([128, K], F32, tag="eout")
                nc.vector.scalar_tensor_tensor(out=eout[:], in0=r2[:],
                                               scalar=c2, in1=ene[:],
                                               op0=OP.is_le, op1=OP.mult)
                nc.sync.dma_start(
                    out=ene_d[bass.ts(t, 128 * K)].rearrange(
                        "(p x) -> p x", x=K),
                    in_=eout[:])
    nc.compile()
    return nc


_NC_CACHE = {}


def _get_nc(builder, key, *args, **kw):
    if key not in _NC_CACHE:
        _NC_CACHE[key] = builder(*args, **kw)
    return _NC_CACHE[key]


def _host_reference(coords, pairs, box, c6, b, cutoff, atom_types):
    # numpy fallback for non-orthorhombic boxes (not hit by the real inputs)
    dr = coords[pairs[:, 1]] - coords[pairs[:, 0]]
    inv_box = np.linalg.inv(box)
    dr = dr - np.round(dr @ inv_box) @ box
    r = np.sqrt((dr * dr).sum(1))
    ti = atom_types[pairs[:, 0]]
    tj = atom_types[pairs[:, 1]]
    u = b[ti, tj] * r
    poly = 1.0 + u * (1.0 + u / 2.0 * (1.0 + u / 3.0 * (1.0 + u / 4.0 *
                     (1.0 + u / 5.0 * (1.0 + u / 6.0)))))
    f6 = 1.0 - np.exp(-u) * poly
    ene = -(c6[ti, tj] * f6) / r ** 6
    return np.where(r <= cutoff, ene, 0.0).astype(np.float32)


def _plane_sets_a(coords_n, pi, pj):
    """Per-core input dicts for kernel A (normalized coord SoA planes).

    The i planes are NEGATED (device computes d = j + (-i)).
    """
    P = pi.shape[0]
    total = N_CORES * SLOTS_A
    planes = {}
    for ci, ax in enumerate(AXES):
        pj_pl = np.full(total, 0.25, np.float32)
        pi_pl = np.zeros(total, np.float32)
        pj_pl[:P] = coords_n[pj, ci]
        pi_pl[:P] = -coords_n[pi, ci]
        planes[f"j{ax}"] = pj_pl
        planes[f"i{ax}"] = pi_pl
    return [{k: v[c * SLOTS_A:(c + 1) * SLOTS_A] for k, v in planes.items()}
            for c in range(N_CORES)]


def _bufs_b(coords_n, c6, b, pi, pj, ti, tj, sl):
    """Kernel B input planes for one chunk of survivor indices `sl`."""
    cap = N_CORES * SLOTS_B
    n = sl.shape[0]
    buf = {}
    for ci, ax in enumerate(AXES):
        jb = np.full(cap, 0.25, np.float32)
        ib = np.zeros(cap, np.float32)
        jb[:n] = coords_n[pj[sl], ci]
        ib[:n] = -coords_n[pi[sl], ci]
        buf[f"j{ax}"] = jb
        buf[f"i{ax}"] = ib
    cpb = np.zeros(cap, np.float32)
    bpb = np.ones(cap, np.float32)
    cpb[:n] = -c6[ti, tj]
    bpb[:n] = b[ti, tj]
    buf["cp"] = cpb
    buf["bp"] = bpb
    return buf


def kernel(coords, pairs, box, c6, b, cutoff, atom_types):
    coords = np.asarray(coords, np.float32)
    pairs = np.asarray(pairs)
    box = np.asarray(box, np.float32)
    c6 = np.asarray(c6, np.float32)
    b = np.asarray(b, np.float32)
    atom_types = np.asarray(atom_types).astype(np.int64)
    cutoff = float(np.asarray(cutoff))

    offdiag = box - np.diag(np.diag(box))
    if np.any(offdiag != 0.0) or pairs.shape[0] > N_CORES * SLOTS_A:
        return _host_reference(coords, pairs, box, c6, b, cutoff, atom_types)
    Ls = tuple(float(box[i, i]) for i in range(3))

    P = pairs.shape[0]
    pi = np.ascontiguousarray(pairs[:, 0]).astype(np.int64)
    pj = np.ascontiguousarray(pairs[:, 1]).astype(np.int64)
    coords_n = coords / np.asarray(Ls, np.float32)[None, :]

    # ---- kernel A: dense in-cutoff screen ----
    nc_a = _get_nc(build_a, ("a", Ls, round(cutoff, 6)), Ls, cutoff)
    in_a = _plane_sets_a(coords_n, pi, pj)
    res_a = run_bass_kernel_spmd(nc_a, in_a, core_ids=list(range(N_CORES)))
    mask = np.concatenate([res_a.results[c]["mask"] for c in range(N_CORES)])

    # ---- host: compact by the device-produced mask (index routing only) ----
    idx = np.flatnonzero(mask[:P])

    # ---- kernel B: exact energies for survivors (chunked if ever needed) ----
    nc_b = _get_nc(build_b, ("b", Ls, round(cutoff, 6)), Ls, cutoff)
    cap = N_CORES * SLOTS_B
    ene_s = np.empty(idx.shape[0], np.float32)
    for lo in range(0, max(idx.shape[0], 1), cap):
        sl = idx[lo:lo + cap]
        ti = atom_types[pi[sl]]
        tj = atom_types[pj[sl]]
        buf = _bufs_b(coords_n, c6, b, pi, pj, ti, tj, sl)
        in_b = [{k: v[c * SLOTS_B:(c + 1) * SLOTS_B] for k, v in buf.items()}
                for c in range(N_CORES)]
        res_b = run_bass_kernel_spmd(nc_b, in_b, core_ids=list(range(N_CORES)))
        ene_full = np.concatenate([res_b.results[c]["ene"]
                                   for c in range(N_CORES)])
        ene_s[lo:lo + sl.shape[0]] = ene_full[:sl.shape[0]]

    out = np.zeros(P, np.float32)
    out[idx] = ene_s
    return out
